# revision 1
# baseline (speedup 1.0000x reference)
"""Windowed multi-head attention (DWAttention) Bass kernel for Trainium2.

Problem: x[B=2, n=64, N=256, C=384] -> per-window MHA (H=12, d=32) with fused
QKV projection + out_proj (no bias on out_proj, in_proj bias provided).

Strategy (8 NeuronCores, data-parallel over the B*n = 128 independent
windows -> 16 windows per core):

Per window w (tokens N=256, channels C=384 = 3 partition-tiles of 128):
  1. Host supplies x^T [C, N] (layout prep on host, analogous to
     pre-transposed weights).  All matmuls use float32r (full-rate fp32).
  2. qk^T = W_qk @ x^T: 6 psum tiles [128, 256] (chan-major), evicted to
     SBUF with per-partition bias add (DVE tensor_scalar).
  3. v = x @ W_v^T: 2 psum tiles [128, 384] (token-major), evicted with
     broadcast bias add.
  4. Per head-group g of 4 heads (3 groups), per k-tile t (2):
     S^T[k, q] = k_h @ q_h^T via row-group packed matmuls (K=d=32, 4 heads
     concurrent in the 128x128 array) -> psum [128, 4*256].
     exp via ScalarE activation (scale=1/sqrt(d) fused), psum -> SBUF.
  5. attn@v + denominator: col-group packed matmuls (M=32 per head):
     o^T[d, q] accumulates over the 2 k-tiles; denominator rows = ones^T
     matmuls producing the k-sum replicated over each head's 32 partitions.
     Normalize during psum->SBUF eviction: oT_sb = psum_oT * recip(den).
  6. out = o @ W_o^T: lhsT = oT tiles (exactly the c-major layout produced
     in 5), 2 psum tiles [128, 384], evicted and DMA'd out.
"""

import numpy as np
from contextlib import ExitStack

import concourse.bass as bass
import concourse.mybir as mybir
import concourse.tile as tile
from concourse import bacc
from concourse.bass_utils import run_bass_kernel_spmd

# Problem constants (hardcoded per contract).
B, NWIN, N, C = 2, 64, 256, 384
H, D = 12, 32
SCALE = float(D) ** -0.5
NCORES = 8
WPC = (B * NWIN) // NCORES  # windows per core = 16
CT = C // 128               # channel tiles = 3
TT = N // 128               # token tiles = 2
NG = H // 4                 # head groups of 4 = 3

F32 = mybir.dt.float32
F32R = mybir.dt.float32r
BF16 = mybir.dt.bfloat16
ADD = mybir.AluOpType.add
MULT = mybir.AluOpType.mult
EXP = mybir.ActivationFunctionType.Exp


def _r(ap):
    """Bitcast an fp32 AP to float32r for full-rate PE matmuls."""
    return ap.bitcast(F32R)


def build_program(stage=4, wpc=WPC, reps=0):
    """stage: 1=proj only, 2=+scores/exp, 3=+attnv/recip, 4=full.

    reps>0 wraps the whole per-core body in a hardware loop executing it
    reps times — used only for wall-clock HW timing (outputs unchanged)."""
    nc = bacc.Bacc()

    xt_h = nc.dram_tensor("xt", [wpc, CT, 128, N], F32R, kind="ExternalInput")
    wqk_h = nc.dram_tensor("wqkt", [CT, 128, 2 * C], F32R, kind="ExternalInput")
    wv_h = nc.dram_tensor("wvt", [CT, 128, C], F32R, kind="ExternalInput")
    wo_h = nc.dram_tensor("wot", [CT, 128, C], F32R, kind="ExternalInput")
    bqk_h = nc.dram_tensor("bqkt", [128, 2 * CT], F32, kind="ExternalInput")
    bvb_h = nc.dram_tensor("bvb", [128, C], F32, kind="ExternalInput")
    out_h = nc.dram_tensor("out", [wpc, TT, 128, C], F32, kind="ExternalOutput")
    dbg_h = None
    if stage == 1:
        dbg_h = nc.dram_tensor("dbg", [wpc, 128, 2 * CT * N], F32R, kind="ExternalOutput")
    elif stage == 2:
        dbg_h = nc.dram_tensor("dbg", [wpc, NG, 128, 8 * N], BF16, kind="ExternalOutput")
    elif stage == 3:
        dbg_h = nc.dram_tensor("dbg", [wpc, NG, 128, N], F32R, kind="ExternalOutput")

    with ExitStack() as ctx:
        tc = ctx.enter_context(tile.TileContext(nc))
        wpool = ctx.enter_context(tc.tile_pool(name="wpool", bufs=1))
        xpool = ctx.enter_context(tc.tile_pool(name="xpool", bufs=4))
        qkpool = ctx.enter_context(tc.tile_pool(name="qkpool", bufs=3))
        vpool = ctx.enter_context(tc.tile_pool(name="vpool", bufs=3))
        apool = ctx.enter_context(tc.tile_pool(name="apool", bufs=3))
        rpool = ctx.enter_context(tc.tile_pool(name="rpool", bufs=4))
        opool = ctx.enter_context(tc.tile_pool(name="opool", bufs=6))
        fpool = ctx.enter_context(tc.tile_pool(name="fpool", bufs=3))
        proj_ps = ctx.enter_context(tc.tile_pool(name="proj_ps", bufs=3, space="PSUM"))
        sc_ps = ctx.enter_context(tc.tile_pool(name="sc_ps", bufs=2, space="PSUM"))
        att_ps = ctx.enter_context(tc.tile_pool(name="att_ps", bufs=1, space="PSUM"))

        # ---- one-time constants ----
        wqk_sb = wpool.tile([128, CT, 2 * C], F32R)
        nc.sync.dma_start(out=wqk_sb, in_=wqk_h.ap().rearrange("c p o -> p c o"))
        wv_sb = wpool.tile([128, CT, C], F32R)
        nc.sync.dma_start(out=wv_sb, in_=wv_h.ap().rearrange("c p o -> p c o"))
        wo_sb = wpool.tile([128, CT, C], F32R)
        nc.sync.dma_start(out=wo_sb, in_=wo_h.ap().rearrange("c p o -> p c o"))
        bqk_sb = wpool.tile([128, 2 * CT], F32)
        nc.sync.dma_start(out=bqk_sb, in_=bqk_h.ap())
        bvb_sb = wpool.tile([128, C], F32)
        nc.sync.dma_start(out=bvb_sb, in_=bvb_h.ap())
        ones_sb = wpool.tile([128, 32], BF16)
        nc.vector.memset(ones_sb, 1.0)

        loop_ctx = tc.For_i(0, reps) if reps else None
        if loop_ctx is not None:
            ctx.enter_context(loop_ctx)
        for w in range(wpc):
            # ---- load x^T for this window ----
            xt_sb = xpool.tile([128, CT, N], F32R)
            nc.sync.dma_start(out=xt_sb, in_=xt_h.ap()[w].rearrange("c p t -> p c t"))

            # ---- qk^T projection: 6 output chan-tiles of [128, 256] ----
            qk_sb = qkpool.tile([128, 2 * CT, N], BF16)
            for j in range(2 * CT):
                ps = proj_ps.tile([128, N], F32, tag="proj", name="ps_qk")
                for c in range(CT):
                    nc.tensor.matmul(
                        ps,
                        wqk_sb[:, c, 128 * j:128 * (j + 1)],
                        xt_sb[:, c, :],
                        start=(c == 0), stop=(c == CT - 1),
                    )
                nc.vector.tensor_scalar(
                    out=qk_sb[:, j, :], in0=ps,
                    scalar1=bqk_sb[:, j:j + 1], scalar2=None, op0=ADD,
                )

            # ---- v projection: 2 token-tiles of [128, 384] ----
            v_sb = vpool.tile([128, TT, C], BF16)
            for m in range(TT):
                ps = proj_ps.tile([128, C], F32, tag="proj", name="ps_v")
                for c in range(CT):
                    nc.tensor.matmul(
                        ps,
                        xt_sb[:, c, 128 * m:128 * (m + 1)],
                        wv_sb[:, c, :],
                        start=(c == 0), stop=(c == CT - 1),
                    )
                nc.vector.tensor_tensor(
                    out=v_sb[:, m, :], in0=ps, in1=bvb_sb, op=ADD,
                )

            if stage == 1:
                nc.sync.dma_start(out=dbg_h.ap()[w], in_=qk_sb.rearrange("p j n -> p (j n)"))
                continue

            # ---- attention per head-group of 4 ----
            ot_tiles = []
            for g in range(NG):
                attn_sb = apool.tile([128, 2 * 4 * N], BF16, name="attn_sb")
                for t in range(TT):
                    for u in range(2):
                        # one PSUM bank per matmul group (HW requirement):
                        # 2-head units of 2 banks so the pool double-buffers
                        scp = sc_ps.tile([128, 2, 512], F32, name="scp")
                        for h2 in range(2):
                            hh = 2 * u + h2
                            # S^T[k_slice, q] = k_h[k_slice] @ q_h^T
                            nc.tensor.matmul(
                                scp[:, h2, 0:N],
                                qk_sb[32 * hh:32 * (hh + 1), CT + g, 128 * t:128 * (t + 1)],
                                qk_sb[32 * hh:32 * (hh + 1), g, :],
                                start=True, stop=True,
                                tile_position=(32 * hh, 0),
                            )
                        nc.scalar.activation(
                            out=attn_sb[:, 4 * N * t + 2 * N * u: 4 * N * t + 2 * N * (u + 1)],
                            in_=scp[:, :, 0:N],
                            func=EXP, scale=SCALE,
                        )

                if stage == 2:
                    nc.sync.dma_start(out=dbg_h.ap()[w][g], in_=attn_sb)
                    continue

                # o^T accumulation bank; denominators borrow a scores-pool slot
                od = att_ps.tile([128, 512], F32, name="od")
                dent = sc_ps.tile([128, 512], F32, tag="scp", name="dent")
                for t in range(TT):
                    for hh in range(4):
                        nc.tensor.matmul(
                            dent[32 * hh:32 * (hh + 1), 0:N],
                            ones_sb[:, :],
                            attn_sb[:, 4 * N * t + N * hh: 4 * N * t + N * (hh + 1)],
                            start=(t == 0), stop=(t == TT - 1),
                            tile_position=(0, 32 * hh),
                            skip_group_check=True,
                        )
                recip_sb = rpool.tile([128, N], F32)
                nc.vector.reciprocal_approx_fast(recip_sb, dent[:, 0:N])
                for t in range(TT):
                    for hh in range(4):
                        h = 4 * g + hh
                        nc.tensor.matmul(
                            od[32 * hh:32 * (hh + 1), 0:N],
                            v_sb[:, t, 32 * h:32 * (h + 1)],
                            attn_sb[:, 4 * N * t + N * hh: 4 * N * t + N * (hh + 1)],
                            start=(t == 0), stop=(t == TT - 1),
                            tile_position=(0, 32 * hh),
                            skip_group_check=True,
                        )
                ot_sb = opool.tile([128, N], F32R, name="ot_sb")
                nc.vector.tensor_tensor(out=ot_sb, in0=od[:, 0:N], in1=recip_sb, op=MULT)
                ot_tiles.append(ot_sb)
                if stage == 3:
                    nc.sync.dma_start(out=dbg_h.ap()[w][g], in_=ot_sb)

            if stage in (2, 3):
                continue

            # ---- out projection ----
            of_sb = fpool.tile([128, TT, C], F32)
            for m in range(TT):
                ps = proj_ps.tile([128, C], F32, tag="proj", name="ps_out")
                for g in range(NG):
                    nc.tensor.matmul(
                        ps,
                        ot_tiles[g][:, 128 * m:128 * (m + 1)],
                        wo_sb[:, g, :],
                        start=(g == 0), stop=(g == NG - 1),
                    )
                if m == 0:
                    nc.scalar.copy(out=of_sb[:, m, :], in_=ps)
                else:
                    nc.vector.tensor_copy(out=of_sb[:, m, :], in_=ps)
            nc.sync.dma_start(out=out_h.ap()[w].rearrange("m p c -> p m c"), in_=of_sb)

    nc.compile()
    return nc


_PROGRAM = None


def _get_program():
    global _PROGRAM
    if _PROGRAM is None:
        _PROGRAM = build_program()
    return _PROGRAM


def make_in_maps(x, in_proj_weight, in_proj_bias, out_proj_weight):
    x = np.asarray(x, dtype=np.float32)
    in_proj_weight = np.asarray(in_proj_weight, dtype=np.float32)
    in_proj_bias = np.asarray(in_proj_bias, dtype=np.float32)
    out_proj_weight = np.asarray(out_proj_weight, dtype=np.float32)

    xt = np.ascontiguousarray(x.reshape(B * NWIN, N, C).transpose(0, 2, 1))
    xt = xt.reshape(NCORES, WPC, CT, 128, N)
    wqkt = np.ascontiguousarray(in_proj_weight[:2 * C].T).reshape(CT, 128, 2 * C)
    wvt = np.ascontiguousarray(in_proj_weight[2 * C:].T).reshape(CT, 128, C)
    wot = np.ascontiguousarray(out_proj_weight.T).reshape(CT, 128, C)
    bqkt = np.ascontiguousarray(in_proj_bias[:2 * C].reshape(2 * CT, 128).T)
    bvb = np.ascontiguousarray(np.broadcast_to(in_proj_bias[2 * C:], (128, C)))
    return [
        {"xt": xt[i], "wqkt": wqkt, "wvt": wvt, "wot": wot, "bqkt": bqkt, "bvb": bvb}
        for i in range(NCORES)
    ]


def assemble_out(results):
    outs = [r["out"].reshape(WPC, N, C) for r in results]
    return np.concatenate(outs).reshape(B, NWIN, N, C).astype(np.float32)


def kernel(x, in_proj_weight, in_proj_bias, out_proj_weight):
    nc = _get_program()
    in_maps = make_in_maps(x, in_proj_weight, in_proj_bias, out_proj_weight)
    res = run_bass_kernel_spmd(nc, in_maps, core_ids=list(range(NCORES)))
    return assemble_out(res.results)



# revision 38
# speedup vs baseline: 20.8121x; 20.8121x over previous
"""Windowed multi-head attention (DWAttention) Bass kernel for Trainium2.

Problem: x[B=2, n=64, N=256, C=384] -> per-window MHA (H=12, d=32) with fused
QKV projection + out_proj (no bias on out_proj, in_proj bias provided).

Strategy (8 NeuronCores, data-parallel over the B*n = 128 independent
windows -> 16 windows per core):

Per window w (tokens N=256, channels C=384 = 3 partition-tiles of 128):
  1. Host supplies x^T [C, N] (layout prep on host, analogous to
     pre-transposed weights).  All matmuls use float32r (full-rate fp32).
  2. qk^T = W_qk @ x^T: 6 psum tiles [128, 256] (chan-major), evicted to
     SBUF with per-partition bias add (DVE tensor_scalar).
  3. v = x @ W_v^T: 2 psum tiles [128, 384] (token-major), evicted with
     broadcast bias add.
  4. Per head-group g of 4 heads (3 groups), per k-tile t (2):
     S^T[k, q] = k_h @ q_h^T via row-group packed matmuls (K=d=32, 4 heads
     concurrent in the 128x128 array) -> psum [128, 4*256].
     exp via ScalarE activation (scale=1/sqrt(d) fused), psum -> SBUF.
  5. attn@v + denominator: col-group packed matmuls (M=32 per head):
     o^T[d, q] accumulates over the 2 k-tiles; denominator rows = ones^T
     matmuls producing the k-sum replicated over each head's 32 partitions.
     Normalize during psum->SBUF eviction: oT_sb = psum_oT * recip(den).
  6. out = o @ W_o^T: lhsT = oT tiles (exactly the c-major layout produced
     in 5), 2 psum tiles [128, 384], evicted and DMA'd out.
"""

import numpy as np
from contextlib import ExitStack

import concourse.bass as bass
import concourse.mybir as mybir
import concourse.tile as tile
from concourse import bacc
from concourse.bass_utils import run_bass_kernel_spmd

# Problem constants (hardcoded per contract).
B, NWIN, N, C = 2, 64, 256, 384
H, D = 12, 32
SCALE = float(D) ** -0.5
NCORES = 8
WPC = (B * NWIN) // NCORES  # windows per core = 16
CT = C // 128               # channel tiles = 3
TT = N // 128               # token tiles = 2
NG = H // 4                 # head groups of 4 = 3

F32 = mybir.dt.float32
F32R = mybir.dt.float32r
BF16 = mybir.dt.bfloat16
ADD = mybir.AluOpType.add
MULT = mybir.AluOpType.mult
EXP = mybir.ActivationFunctionType.Exp


def _r(ap):
    """Bitcast an fp32 AP to float32r for full-rate PE matmuls."""
    return ap.bitcast(F32R)


def build_program(stage=4, wpc=WPC, reps=0):
    """stage: 1=proj only, 2=+scores/exp, 3=+attnv/recip, 4=full.

    reps>0 wraps the whole per-core body in a hardware loop executing it
    reps times — used only for wall-clock HW timing (outputs unchanged)."""
    nc = bacc.Bacc()

    xt_h = nc.dram_tensor("xt", [wpc, CT, 128, N], F32R, kind="ExternalInput")
    wqk_h = nc.dram_tensor("wqkt", [CT, 128, 2 * C], F32R, kind="ExternalInput")
    wv_h = nc.dram_tensor("wvt", [CT, 128, C], F32R, kind="ExternalInput")
    wo_h = nc.dram_tensor("wot", [CT, 128, C], F32R, kind="ExternalInput")
    bqk_h = nc.dram_tensor("bqkt", [128, 2 * CT], F32, kind="ExternalInput")
    bvb_h = nc.dram_tensor("bvb", [128, C], F32, kind="ExternalInput")
    out_h = nc.dram_tensor("out", [wpc, TT, 128, C], F32, kind="ExternalOutput")
    dbg_h = None
    if stage == 1:
        dbg_h = nc.dram_tensor("dbg", [wpc, 128, 2 * CT * N], F32R, kind="ExternalOutput")
    elif stage == 2:
        dbg_h = nc.dram_tensor("dbg", [wpc, NG, 128, 8 * N], BF16, kind="ExternalOutput")
    elif stage == 3:
        dbg_h = nc.dram_tensor("dbg", [wpc, NG, 128, N], F32R, kind="ExternalOutput")

    with ExitStack() as ctx:
        tc = ctx.enter_context(tile.TileContext(nc))
        wpool = ctx.enter_context(tc.tile_pool(name="wpool", bufs=1))
        xpool = ctx.enter_context(tc.tile_pool(name="xpool", bufs=4))
        qkpool = ctx.enter_context(tc.tile_pool(name="qkpool", bufs=3))
        vpool = ctx.enter_context(tc.tile_pool(name="vpool", bufs=3))
        apool = ctx.enter_context(tc.tile_pool(name="apool", bufs=3))
        rpool = ctx.enter_context(tc.tile_pool(name="rpool", bufs=4))
        opool = ctx.enter_context(tc.tile_pool(name="opool", bufs=6))
        fpool = ctx.enter_context(tc.tile_pool(name="fpool", bufs=3))
        proj_ps = ctx.enter_context(tc.tile_pool(name="proj_ps", bufs=3, space="PSUM"))
        sc_ps = ctx.enter_context(tc.tile_pool(name="sc_ps", bufs=2, space="PSUM"))
        att_ps = ctx.enter_context(tc.tile_pool(name="att_ps", bufs=1, space="PSUM"))

        # ---- one-time constants ----
        wqk_sb = wpool.tile([128, CT, 2 * C], F32R)
        nc.sync.dma_start(out=wqk_sb, in_=wqk_h.ap().rearrange("c p o -> p c o"))
        wv_sb = wpool.tile([128, CT, C], F32R)
        nc.sync.dma_start(out=wv_sb, in_=wv_h.ap().rearrange("c p o -> p c o"))
        wo_sb = wpool.tile([128, CT, C], F32R)
        nc.sync.dma_start(out=wo_sb, in_=wo_h.ap().rearrange("c p o -> p c o"))
        bqk_sb = wpool.tile([128, 2 * CT], F32)
        nc.sync.dma_start(out=bqk_sb, in_=bqk_h.ap())
        bvb_sb = wpool.tile([128, C], F32)
        nc.sync.dma_start(out=bvb_sb, in_=bvb_h.ap())
        ones_sb = wpool.tile([128, 32], BF16)
        nc.vector.memset(ones_sb, 1.0)

        loop_ctx = tc.For_i(0, reps) if reps else None
        if loop_ctx is not None:
            ctx.enter_context(loop_ctx)
        for w in range(wpc):
            # ---- load x^T for this window ----
            xt_sb = xpool.tile([128, CT, N], F32R)
            nc.sync.dma_start(out=xt_sb, in_=xt_h.ap()[w].rearrange("c p t -> p c t"))

            # ---- qk^T projection: 6 output chan-tiles of [128, 256] ----
            qk_sb = qkpool.tile([128, 2 * CT, N], BF16)
            for j in range(2 * CT):
                ps = proj_ps.tile([128, N], F32, tag="proj", name="ps_qk")
                for c in range(CT):
                    nc.tensor.matmul(
                        ps,
                        wqk_sb[:, c, 128 * j:128 * (j + 1)],
                        xt_sb[:, c, :],
                        start=(c == 0), stop=(c == CT - 1),
                    )
                nc.vector.tensor_scalar(
                    out=qk_sb[:, j, :], in0=ps,
                    scalar1=bqk_sb[:, j:j + 1], scalar2=None, op0=ADD,
                )

            # ---- v projection: 2 token-tiles of [128, 384] ----
            v_sb = vpool.tile([128, TT, C], BF16)
            for m in range(TT):
                ps = proj_ps.tile([128, C], F32, tag="proj", name="ps_v")
                for c in range(CT):
                    nc.tensor.matmul(
                        ps,
                        xt_sb[:, c, 128 * m:128 * (m + 1)],
                        wv_sb[:, c, :],
                        start=(c == 0), stop=(c == CT - 1),
                    )
                nc.vector.tensor_tensor(
                    out=v_sb[:, m, :], in0=ps, in1=bvb_sb, op=ADD,
                )

            if stage == 1:
                nc.sync.dma_start(out=dbg_h.ap()[w], in_=qk_sb.rearrange("p j n -> p (j n)"))
                continue

            # ---- attention per head-group of 4 ----
            ot_tiles = []
            for g in range(NG):
                attn_sb = apool.tile([128, 2 * 4 * N], BF16, name="attn_sb")
                for t in range(TT):
                    for u in range(2):
                        # one PSUM bank per matmul group (HW requirement):
                        # 2-head units of 2 banks so the pool double-buffers
                        scp = sc_ps.tile([128, 2, 512], F32, name="scp")
                        for h2 in range(2):
                            hh = 2 * u + h2
                            # S^T[k_slice, q] = k_h[k_slice] @ q_h^T
                            nc.tensor.matmul(
                                scp[:, h2, 0:N],
                                qk_sb[32 * hh:32 * (hh + 1), CT + g, 128 * t:128 * (t + 1)],
                                qk_sb[32 * hh:32 * (hh + 1), g, :],
                                start=True, stop=True,
                                tile_position=(32 * hh, 0),
                            )
                        nc.scalar.activation(
                            out=attn_sb[:, 4 * N * t + 2 * N * u: 4 * N * t + 2 * N * (u + 1)],
                            in_=scp[:, :, 0:N],
                            func=EXP, scale=SCALE,
                        )

                if stage == 2:
                    nc.sync.dma_start(out=dbg_h.ap()[w][g], in_=attn_sb)
                    continue

                # o^T accumulation bank; denominators borrow a scores-pool slot
                od = att_ps.tile([128, 512], F32, name="od")
                dent = sc_ps.tile([128, 512], F32, tag="scp", name="dent")
                for t in range(TT):
                    for hh in range(4):
                        nc.tensor.matmul(
                            dent[32 * hh:32 * (hh + 1), 0:N],
                            ones_sb[:, :],
                            attn_sb[:, 4 * N * t + N * hh: 4 * N * t + N * (hh + 1)],
                            start=(t == 0), stop=(t == TT - 1),
                            tile_position=(0, 32 * hh),
                            skip_group_check=True,
                        )
                recip_sb = rpool.tile([128, N], F32)
                nc.vector.reciprocal_approx_fast(recip_sb, dent[:, 0:N])
                for t in range(TT):
                    for hh in range(4):
                        h = 4 * g + hh
                        nc.tensor.matmul(
                            od[32 * hh:32 * (hh + 1), 0:N],
                            v_sb[:, t, 32 * h:32 * (h + 1)],
                            attn_sb[:, 4 * N * t + N * hh: 4 * N * t + N * (hh + 1)],
                            start=(t == 0), stop=(t == TT - 1),
                            tile_position=(0, 32 * hh),
                            skip_group_check=True,
                        )
                ot_sb = opool.tile([128, N], F32R, name="ot_sb")
                nc.vector.tensor_tensor(out=ot_sb, in0=od[:, 0:N], in1=recip_sb, op=MULT)
                ot_tiles.append(ot_sb)
                if stage == 3:
                    nc.sync.dma_start(out=dbg_h.ap()[w][g], in_=ot_sb)

            if stage in (2, 3):
                continue

            # ---- out projection ----
            of_sb = fpool.tile([128, TT, C], F32)
            for m in range(TT):
                ps = proj_ps.tile([128, C], F32, tag="proj", name="ps_out")
                for g in range(NG):
                    nc.tensor.matmul(
                        ps,
                        ot_tiles[g][:, 128 * m:128 * (m + 1)],
                        wo_sb[:, g, :],
                        start=(g == 0), stop=(g == NG - 1),
                    )
                if m == 0:
                    nc.scalar.copy(out=of_sb[:, m, :], in_=ps)
                else:
                    nc.vector.tensor_copy(out=of_sb[:, m, :], in_=ps)
            nc.sync.dma_start(out=out_h.ap()[w].rearrange("m p c -> p m c"), in_=of_sb)

    nc.compile()
    return nc


_PROGRAM = None


def _get_program():
    global _PROGRAM
    if _PROGRAM is None:
        _PROGRAM = build_program()
    return _PROGRAM


def make_in_maps(x, in_proj_weight, in_proj_bias, out_proj_weight):
    x = np.asarray(x, dtype=np.float32)
    in_proj_weight = np.asarray(in_proj_weight, dtype=np.float32)
    in_proj_bias = np.asarray(in_proj_bias, dtype=np.float32)
    out_proj_weight = np.asarray(out_proj_weight, dtype=np.float32)

    xt = np.ascontiguousarray(x.reshape(B * NWIN, N, C).transpose(0, 2, 1))
    xt = xt.reshape(NCORES, WPC, CT, 128, N)
    wqkt = np.ascontiguousarray(in_proj_weight[:2 * C].T).reshape(CT, 128, 2 * C)
    wvt = np.ascontiguousarray(in_proj_weight[2 * C:].T).reshape(CT, 128, C)
    wot = np.ascontiguousarray(out_proj_weight.T).reshape(CT, 128, C)
    bqkt = np.ascontiguousarray(in_proj_bias[:2 * C].reshape(2 * CT, 128).T)
    bvb = np.ascontiguousarray(np.broadcast_to(in_proj_bias[2 * C:], (128, C)))
    return [
        {"xt": xt[i], "wqkt": wqkt, "wvt": wvt, "wot": wot, "bqkt": bqkt, "bvb": bvb}
        for i in range(NCORES)
    ]


def assemble_out(results):
    outs = [r["out"].reshape(WPC, N, C) for r in results]
    return np.concatenate(outs).reshape(B, NWIN, N, C).astype(np.float32)


def kernel(x, in_proj_weight, in_proj_bias, out_proj_weight):
    nc = _get_program()
    in_maps = make_in_maps(x, in_proj_weight, in_proj_bias, out_proj_weight)
    res = run_bass_kernel_spmd(nc, in_maps, core_ids=list(range(NCORES)))
    return assemble_out(res.results)



# revision 39
# speedup vs baseline: 22.8365x; 1.0973x over previous
"""Windowed multi-head attention (DWAttention) Bass kernel for Trainium2.

Problem: x[B=2, n=64, N=256, C=384] -> per-window MHA (H=12, d=32) with fused
QKV projection + out_proj (no bias on out_proj, in_proj bias provided).

Strategy (8 NeuronCores, data-parallel over the B*n = 128 independent
windows -> 16 windows per core):

Per window w (tokens N=256, channels C=384 = 3 partition-tiles of 128):
  1. Host supplies x^T [C, N] (layout prep on host, analogous to
     pre-transposed weights).  All matmuls use float32r (full-rate fp32).
  2. qk^T = W_qk @ x^T: 6 psum tiles [128, 256] (chan-major), evicted to
     SBUF with per-partition bias add (DVE tensor_scalar).
  3. v = x @ W_v^T: 2 psum tiles [128, 384] (token-major), evicted with
     broadcast bias add.
  4. Per head-group g of 4 heads (3 groups), per k-tile t (2):
     S^T[k, q] = k_h @ q_h^T via row-group packed matmuls (K=d=32, 4 heads
     concurrent in the 128x128 array) -> psum [128, 4*256].
     exp via ScalarE activation (scale=1/sqrt(d) fused), psum -> SBUF.
  5. attn@v + denominator: col-group packed matmuls (M=32 per head):
     o^T[d, q] accumulates over the 2 k-tiles; denominator rows = ones^T
     matmuls producing the k-sum replicated over each head's 32 partitions.
     Normalize during psum->SBUF eviction: oT_sb = psum_oT * recip(den).
  6. out = o @ W_o^T: lhsT = oT tiles (exactly the c-major layout produced
     in 5), 2 psum tiles [128, 384], evicted and DMA'd out.
"""

import numpy as np
from contextlib import ExitStack

import concourse.bass as bass
import concourse.mybir as mybir
import concourse.tile as tile
from concourse import bacc
from concourse.bass_utils import run_bass_kernel_spmd

# Problem constants (hardcoded per contract).
B, NWIN, N, C = 2, 64, 256, 384
H, D = 12, 32
SCALE = float(D) ** -0.5
NCORES = 8
WPC = (B * NWIN) // NCORES  # windows per core = 16
CT = C // 128               # channel tiles = 3
TT = N // 128               # token tiles = 2
NG = H // 4                 # head groups of 4 = 3

F32 = mybir.dt.float32
F32R = mybir.dt.float32r
BF16 = mybir.dt.bfloat16
ADD = mybir.AluOpType.add
MULT = mybir.AluOpType.mult
EXP = mybir.ActivationFunctionType.Exp


def _r(ap):
    """Bitcast an fp32 AP to float32r for full-rate PE matmuls."""
    return ap.bitcast(F32R)


def build_program(stage=4, wpc=WPC, reps=0):
    """stage: 1=proj only, 2=+scores/exp, 3=+attnv/recip, 4=full.

    reps>0 wraps the whole per-core body in a hardware loop executing it
    reps times — used only for wall-clock HW timing (outputs unchanged)."""
    nc = bacc.Bacc()

    xt_h = nc.dram_tensor("xt", [wpc, CT, 128, N], F32R, kind="ExternalInput")
    wqk_h = nc.dram_tensor("wqkt", [CT, 128, 2 * C], F32R, kind="ExternalInput")
    wv_h = nc.dram_tensor("wvt", [CT, 128, C], F32R, kind="ExternalInput")
    wo_h = nc.dram_tensor("wot", [CT, 128, C], F32R, kind="ExternalInput")
    bqk_h = nc.dram_tensor("bqkt", [128, 2 * CT], F32, kind="ExternalInput")
    bvb_h = nc.dram_tensor("bvb", [128, C], F32, kind="ExternalInput")
    out_h = nc.dram_tensor("out", [wpc, TT, 128, C], F32, kind="ExternalOutput")
    dbg_h = None
    if stage == 1:
        dbg_h = nc.dram_tensor("dbg", [wpc, 128, 2 * CT * N], F32R, kind="ExternalOutput")
    elif stage == 2:
        dbg_h = nc.dram_tensor("dbg", [wpc, NG, 128, 8 * N], BF16, kind="ExternalOutput")
    elif stage == 3:
        dbg_h = nc.dram_tensor("dbg", [wpc, NG, 128, N], F32R, kind="ExternalOutput")

    with ExitStack() as ctx:
        tc = ctx.enter_context(tile.TileContext(nc))
        wpool = ctx.enter_context(tc.tile_pool(name="wpool", bufs=1))
        xpool = ctx.enter_context(tc.tile_pool(name="xpool", bufs=4))
        qkpool = ctx.enter_context(tc.tile_pool(name="qkpool", bufs=3))
        vpool = ctx.enter_context(tc.tile_pool(name="vpool", bufs=3))
        apool = ctx.enter_context(tc.tile_pool(name="apool", bufs=3))
        rpool = ctx.enter_context(tc.tile_pool(name="rpool", bufs=4))
        opool = ctx.enter_context(tc.tile_pool(name="opool", bufs=6))
        fpool = ctx.enter_context(tc.tile_pool(name="fpool", bufs=3))
        proj_ps = ctx.enter_context(tc.tile_pool(name="proj_ps", bufs=3, space="PSUM"))
        sc_ps = ctx.enter_context(tc.tile_pool(name="sc_ps", bufs=2, space="PSUM"))
        att_ps = ctx.enter_context(tc.tile_pool(name="att_ps", bufs=1, space="PSUM"))

        # ---- one-time constants ----
        wqk_sb = wpool.tile([128, CT, 2 * C], F32R)
        nc.sync.dma_start(out=wqk_sb, in_=wqk_h.ap().rearrange("c p o -> p c o"))
        wv_sb = wpool.tile([128, CT, C], F32R)
        nc.sync.dma_start(out=wv_sb, in_=wv_h.ap().rearrange("c p o -> p c o"))
        wo_sb = wpool.tile([128, CT, C], F32R)
        nc.sync.dma_start(out=wo_sb, in_=wo_h.ap().rearrange("c p o -> p c o"))
        bqk_sb = wpool.tile([128, 2 * CT], F32)
        nc.sync.dma_start(out=bqk_sb, in_=bqk_h.ap())
        bvb_sb = wpool.tile([128, C], F32)
        nc.sync.dma_start(out=bvb_sb, in_=bvb_h.ap())
        ones_sb = wpool.tile([128, 32], BF16)
        nc.vector.memset(ones_sb, 1.0)

        loop_ctx = tc.For_i(0, reps) if reps else None
        if loop_ctx is not None:
            ctx.enter_context(loop_ctx)
        prev_ots = None
        prev_w = None
        for w in range(wpc):
            # ---- load x^T for this window ----
            xt_sb = xpool.tile([128, CT, N], F32R)
            nc.sync.dma_start(out=xt_sb, in_=xt_h.ap()[w].rearrange("c p t -> p c t"))

            # ---- qk^T projection: 6 output chan-tiles of [128, 256] ----
            qk_sb = qkpool.tile([128, 2 * CT, N], BF16)
            for j in range(2 * CT):
                ps = proj_ps.tile([128, N], F32, tag="proj", name="ps_qk")
                for c in range(CT):
                    nc.tensor.matmul(
                        ps,
                        wqk_sb[:, c, 128 * j:128 * (j + 1)],
                        xt_sb[:, c, :],
                        start=(c == 0), stop=(c == CT - 1),
                    )
                nc.vector.tensor_scalar(
                    out=qk_sb[:, j, :], in0=ps,
                    scalar1=bqk_sb[:, j:j + 1], scalar2=None, op0=ADD,
                )

            # ---- v projection: 2 token-tiles of [128, 384] ----
            v_sb = vpool.tile([128, TT, C], BF16)
            for m in range(TT):
                ps = proj_ps.tile([128, C], F32, tag="proj", name="ps_v")
                for c in range(CT):
                    nc.tensor.matmul(
                        ps,
                        xt_sb[:, c, 128 * m:128 * (m + 1)],
                        wv_sb[:, c, :],
                        start=(c == 0), stop=(c == CT - 1),
                    )
                nc.vector.tensor_tensor(
                    out=v_sb[:, m, :], in0=ps, in1=bvb_sb, op=ADD,
                )

            if stage == 1:
                nc.sync.dma_start(out=dbg_h.ap()[w], in_=qk_sb.rearrange("p j n -> p (j n)"))
                continue

            # ---- attention, head-groups software-pipelined; the previous
            # window's out_proj slots between attnv(g1) and attnv(g2) so PE
            # has work while exp(g2) drains ----
            def do_scores(g):
                attn_sb = apool.tile([128, 2 * 4 * N], BF16, name="attn_sb")
                for t in range(TT):
                    for u in range(2):
                        # one PSUM bank per matmul group (HW requirement):
                        # 2-head units of 2 banks so the pool double-buffers
                        scp = sc_ps.tile([128, 2, 512], F32, name="scp")
                        for h2 in range(2):
                            hh = 2 * u + h2
                            # S^T[k_slice, q] = k_h[k_slice] @ q_h^T
                            nc.tensor.matmul(
                                scp[:, h2, 0:N],
                                qk_sb[32 * hh:32 * (hh + 1), CT + g, 128 * t:128 * (t + 1)],
                                qk_sb[32 * hh:32 * (hh + 1), g, :],
                                start=True, stop=True,
                                tile_position=(32 * hh, 0),
                            )
                        nc.scalar.activation(
                            out=attn_sb[:, 4 * N * t + 2 * N * u: 4 * N * t + 2 * N * (u + 1)],
                            in_=scp[:, :, 0:N],
                            func=EXP, scale=SCALE,
                        )
                return attn_sb

            def do_attnv(g, attn_sb):
                # o^T accumulation bank; denominators borrow a scores-pool slot
                od = att_ps.tile([128, 512], F32, name="od")
                dent = sc_ps.tile([128, 512], F32, tag="scp", name="dent")
                for t in range(TT):
                    for hh in range(4):
                        nc.tensor.matmul(
                            dent[32 * hh:32 * (hh + 1), 0:N],
                            ones_sb[:, :],
                            attn_sb[:, 4 * N * t + N * hh: 4 * N * t + N * (hh + 1)],
                            start=(t == 0), stop=(t == TT - 1),
                            tile_position=(0, 32 * hh),
                            skip_group_check=True,
                        )
                recip_sb = rpool.tile([128, N], F32)
                nc.vector.reciprocal_approx_fast(recip_sb, dent[:, 0:N])
                for t in range(TT):
                    for hh in range(4):
                        h = 4 * g + hh
                        nc.tensor.matmul(
                            od[32 * hh:32 * (hh + 1), 0:N],
                            v_sb[:, t, 32 * h:32 * (h + 1)],
                            attn_sb[:, 4 * N * t + N * hh: 4 * N * t + N * (hh + 1)],
                            start=(t == 0), stop=(t == TT - 1),
                            tile_position=(0, 32 * hh),
                            skip_group_check=True,
                        )
                ot_sb = opool.tile([128, N], F32R, name="ot_sb")
                nc.vector.tensor_tensor(out=ot_sb, in0=od[:, 0:N], in1=recip_sb, op=MULT)
                return ot_sb

            def emit_out_proj(ots, ow):
                of_sb = fpool.tile([128, TT, C], F32)
                for m in range(TT):
                    ps = proj_ps.tile([128, C], F32, tag="proj", name="ps_out")
                    for g in range(NG):
                        nc.tensor.matmul(
                            ps,
                            ots[g][:, 128 * m:128 * (m + 1)],
                            wo_sb[:, g, :],
                            start=(g == 0), stop=(g == NG - 1),
                        )
                    if m == 0:
                        nc.scalar.copy(out=of_sb[:, m, :], in_=ps)
                    else:
                        nc.vector.tensor_copy(out=of_sb[:, m, :], in_=ps)
                nc.sync.dma_start(out=out_h.ap()[ow].rearrange("m p c -> p m c"),
                                  in_=of_sb)

            ot_tiles = []
            attn_next = do_scores(0)
            for g in range(NG):
                attn_cur = attn_next
                if g + 1 < NG:
                    attn_next = do_scores(g + 1)
                if g == NG - 1 and prev_ots is not None and stage == 4:
                    emit_out_proj(prev_ots, prev_w)
                if stage == 2:
                    nc.sync.dma_start(out=dbg_h.ap()[w][g], in_=attn_cur)
                    continue
                ot_sb = do_attnv(g, attn_cur)
                ot_tiles.append(ot_sb)
                if stage == 3:
                    nc.sync.dma_start(out=dbg_h.ap()[w][g], in_=ot_sb)

            if stage in (2, 3):
                continue
            prev_ots = ot_tiles
            prev_w = w

        # ---- out_proj of the final window ----
        if stage == 4:
            of_sb = fpool.tile([128, TT, C], F32)
            for m in range(TT):
                ps = proj_ps.tile([128, C], F32, tag="proj", name="ps_out")
                for g in range(NG):
                    nc.tensor.matmul(
                        ps,
                        prev_ots[g][:, 128 * m:128 * (m + 1)],
                        wo_sb[:, g, :],
                        start=(g == 0), stop=(g == NG - 1),
                    )
                if m == 0:
                    nc.scalar.copy(out=of_sb[:, m, :], in_=ps)
                else:
                    nc.vector.tensor_copy(out=of_sb[:, m, :], in_=ps)
            nc.sync.dma_start(out=out_h.ap()[prev_w].rearrange("m p c -> p m c"),
                              in_=of_sb)

    nc.compile()
    return nc


_PROGRAM = None


def _get_program():
    global _PROGRAM
    if _PROGRAM is None:
        _PROGRAM = build_program()
    return _PROGRAM


def make_in_maps(x, in_proj_weight, in_proj_bias, out_proj_weight):
    x = np.asarray(x, dtype=np.float32)
    in_proj_weight = np.asarray(in_proj_weight, dtype=np.float32)
    in_proj_bias = np.asarray(in_proj_bias, dtype=np.float32)
    out_proj_weight = np.asarray(out_proj_weight, dtype=np.float32)

    xt = np.ascontiguousarray(x.reshape(B * NWIN, N, C).transpose(0, 2, 1))
    xt = xt.reshape(NCORES, WPC, CT, 128, N)
    wqkt = np.ascontiguousarray(in_proj_weight[:2 * C].T).reshape(CT, 128, 2 * C)
    wvt = np.ascontiguousarray(in_proj_weight[2 * C:].T).reshape(CT, 128, C)
    wot = np.ascontiguousarray(out_proj_weight.T).reshape(CT, 128, C)
    bqkt = np.ascontiguousarray(in_proj_bias[:2 * C].reshape(2 * CT, 128).T)
    bvb = np.ascontiguousarray(np.broadcast_to(in_proj_bias[2 * C:], (128, C)))
    return [
        {"xt": xt[i], "wqkt": wqkt, "wvt": wvt, "wot": wot, "bqkt": bqkt, "bvb": bvb}
        for i in range(NCORES)
    ]


def assemble_out(results):
    outs = [r["out"].reshape(WPC, N, C) for r in results]
    return np.concatenate(outs).reshape(B, NWIN, N, C).astype(np.float32)


def kernel(x, in_proj_weight, in_proj_bias, out_proj_weight):
    nc = _get_program()
    in_maps = make_in_maps(x, in_proj_weight, in_proj_bias, out_proj_weight)
    res = run_bass_kernel_spmd(nc, in_maps, core_ids=list(range(NCORES)))
    return assemble_out(res.results)



# revision 41
# speedup vs baseline: 23.9841x; 1.0503x over previous
"""Windowed multi-head attention (DWAttention) Bass kernel for Trainium2.

Problem: x[B=2, n=64, N=256, C=384] -> per-window MHA (H=12, d=32) with fused
QKV projection + out_proj (no bias on out_proj, in_proj bias provided).

Strategy (8 NeuronCores, data-parallel over the B*n = 128 independent
windows -> 16 windows per core):

Per window w (tokens N=256, channels C=384 = 3 partition-tiles of 128):
  1. Host supplies x^T [C, N] (layout prep on host, analogous to
     pre-transposed weights).  All matmuls use float32r (full-rate fp32).
  2. qk^T = W_qk @ x^T: 6 psum tiles [128, 256] (chan-major), evicted to
     SBUF with per-partition bias add (DVE tensor_scalar).
  3. v = x @ W_v^T: 2 psum tiles [128, 384] (token-major), evicted with
     broadcast bias add.
  4. Per head-group g of 4 heads (3 groups), per k-tile t (2):
     S^T[k, q] = k_h @ q_h^T via row-group packed matmuls (K=d=32, 4 heads
     concurrent in the 128x128 array) -> psum [128, 4*256].
     exp via ScalarE activation (scale=1/sqrt(d) fused), psum -> SBUF.
  5. attn@v + denominator: col-group packed matmuls (M=32 per head):
     o^T[d, q] accumulates over the 2 k-tiles; denominator rows = ones^T
     matmuls producing the k-sum replicated over each head's 32 partitions.
     Normalize during psum->SBUF eviction: oT_sb = psum_oT * recip(den).
  6. out = o @ W_o^T: lhsT = oT tiles (exactly the c-major layout produced
     in 5), 2 psum tiles [128, 384], evicted and DMA'd out.

Pipelining: head-groups are software-pipelined (scores/exp of group g+1
issue before attn@v of group g so PE never waits on the Scalar-engine
exp), and each window's out_proj is deferred into the next window's
attention phase (between attnv(g1) and attnv(g2)) so PE has work while
the normalize chain (recip+mult on DVE) drains.
"""

import numpy as np
from contextlib import ExitStack

import concourse.bass as bass
import concourse.mybir as mybir
import concourse.tile as tile
from concourse import bacc
from concourse.bass_utils import run_bass_kernel_spmd

# Problem constants (hardcoded per contract).
B, NWIN, N, C = 2, 64, 256, 384
H, D = 12, 32
SCALE = float(D) ** -0.5
NCORES = 8
WPC = (B * NWIN) // NCORES  # windows per core = 16
CT = C // 128               # channel tiles = 3
TT = N // 128               # token tiles = 2
NG = H // 4                 # head groups of 4 = 3

F32 = mybir.dt.float32
F32R = mybir.dt.float32r
BF16 = mybir.dt.bfloat16
ADD = mybir.AluOpType.add
MULT = mybir.AluOpType.mult
EXP = mybir.ActivationFunctionType.Exp


def _r(ap):
    """Bitcast an fp32 AP to float32r for full-rate PE matmuls."""
    return ap.bitcast(F32R)


def build_program(stage=4, wpc=WPC, reps=0):
    """stage: 1=proj only, 2=+scores/exp, 3=+attnv/recip, 4=full.

    reps>0 wraps the whole per-core body in a hardware loop executing it
    reps times — used only for wall-clock HW timing (outputs unchanged)."""
    nc = bacc.Bacc()

    xt_h = nc.dram_tensor("xt", [wpc, CT, 128, N], F32R, kind="ExternalInput")
    wqk_h = nc.dram_tensor("wqkt", [CT, 128, 2 * C], F32R, kind="ExternalInput")
    wv_h = nc.dram_tensor("wvt", [CT, 128, C], F32R, kind="ExternalInput")
    wo_h = nc.dram_tensor("wot", [CT, 128, C], F32R, kind="ExternalInput")
    bqk_h = nc.dram_tensor("bqkt", [128, 2 * CT, N], F32, kind="ExternalInput")
    bvb_h = nc.dram_tensor("bvb", [128, C], F32, kind="ExternalInput")
    out_h = nc.dram_tensor("out", [wpc, TT, 128, C], F32, kind="ExternalOutput")
    dbg_h = None
    if stage == 1:
        dbg_h = nc.dram_tensor("dbg", [wpc, 128, 2 * CT * N], F32R, kind="ExternalOutput")
    elif stage == 2:
        dbg_h = nc.dram_tensor("dbg", [wpc, NG, 128, 8 * N], BF16, kind="ExternalOutput")
    elif stage == 3:
        dbg_h = nc.dram_tensor("dbg", [wpc, NG, 128, N], F32R, kind="ExternalOutput")

    with ExitStack() as ctx:
        tc = ctx.enter_context(tile.TileContext(nc))
        wpool = ctx.enter_context(tc.tile_pool(name="wpool", bufs=1))
        xpool = ctx.enter_context(tc.tile_pool(name="xpool", bufs=4))
        qkpool = ctx.enter_context(tc.tile_pool(name="qkpool", bufs=3))
        vpool = ctx.enter_context(tc.tile_pool(name="vpool", bufs=3))
        apool = ctx.enter_context(tc.tile_pool(name="apool", bufs=3))
        rpool = ctx.enter_context(tc.tile_pool(name="rpool", bufs=4))
        opool = ctx.enter_context(tc.tile_pool(name="opool", bufs=6))
        fpool = ctx.enter_context(tc.tile_pool(name="fpool", bufs=3))
        proj_ps = ctx.enter_context(tc.tile_pool(name="proj_ps", bufs=3, space="PSUM"))
        sc_ps = ctx.enter_context(tc.tile_pool(name="sc_ps", bufs=2, space="PSUM"))
        att_ps = ctx.enter_context(tc.tile_pool(name="att_ps", bufs=1, space="PSUM"))

        # ---- one-time constants.  DMA transfers drain in issue order, so
        # the first window's critical inputs (wqk q-half chunk 0, xt[0])
        # must land before the weights needed later; when reps==0 the late
        # group issues inside the w==0 body, right after xt[0]. ----
        wqk_sb = wpool.tile([128, CT, 2 * C], F32R)
        wqk_r = wqk_h.ap().rearrange("c p o -> p c o")
        nc.sync.dma_start(out=wqk_sb[:, 0, 0:C], in_=wqk_r[:, 0, 0:C])
        bqk_sb = wpool.tile([128, 2 * CT, N], F32)
        bvb_sb = wpool.tile([128, C], F32)
        wv_sb = wpool.tile([128, CT, C], F32R)
        wo_sb = wpool.tile([128, CT, C], F32R)

        def emit_late_const_dmas():
            nc.sync.dma_start(out=wqk_sb[:, 1, 0:C], in_=wqk_r[:, 1, 0:C])
            nc.sync.dma_start(out=wqk_sb[:, 2, 0:C], in_=wqk_r[:, 2, 0:C])
            nc.sync.dma_start(out=bqk_sb, in_=bqk_h.ap())
            nc.sync.dma_start(out=wqk_sb[:, :, C:2 * C], in_=wqk_r[:, :, C:2 * C])
            nc.sync.dma_start(out=wv_sb, in_=wv_h.ap().rearrange("c p o -> p c o"))
            nc.sync.dma_start(out=bvb_sb, in_=bvb_h.ap())
            nc.sync.dma_start(out=wo_sb, in_=wo_h.ap().rearrange("c p o -> p c o"))

        if reps:
            emit_late_const_dmas()
        ones_sb = wpool.tile([128, 32], BF16)
        nc.vector.memset(ones_sb, 1.0)

        loop_ctx = tc.For_i(0, reps) if reps else None
        if loop_ctx is not None:
            ctx.enter_context(loop_ctx)
        prev_ots = None
        prev_w = None
        for w in range(wpc):
            # ---- load x^T for this window ----
            xt_sb = xpool.tile([128, CT, N], F32R)
            nc.sync.dma_start(out=xt_sb, in_=xt_h.ap()[w].rearrange("c p t -> p c t"))
            if w == 0 and not reps:
                emit_late_const_dmas()

            # ---- qk^T projection: 6 output chan-tiles of [128, 256],
            # paired two-per-PSUM-bank (the pair's first matmul start=True
            # zeroes the whole bank; the second j accumulates start=False
            # onto its pre-zeroed half), one paired eviction ----
            qk_sb = qkpool.tile([128, 2 * CT, N], BF16)
            for p in range(CT):
                ps = proj_ps.tile([128, 2, N], F32, tag="proj", name="ps_qk")
                for jj in range(2):
                    j = 2 * p + jj
                    for c in range(CT):
                        nc.tensor.matmul(
                            ps[:, jj, :],
                            wqk_sb[:, c, 128 * j:128 * (j + 1)],
                            xt_sb[:, c, :],
                            start=(jj == 0 and c == 0), stop=(c == CT - 1),
                            skip_group_check=True,
                        )
                nc.vector.tensor_tensor(
                    out=qk_sb[:, 2 * p:2 * p + 2, :], in0=ps,
                    in1=bqk_sb[:, 2 * p:2 * p + 2, :], op=ADD,
                )

            # ---- v projection: 2 token-tiles of [128, 384] ----
            v_sb = vpool.tile([128, TT, C], BF16)
            for m in range(TT):
                ps = proj_ps.tile([128, C], F32, tag="proj", name="ps_v")
                for c in range(CT):
                    nc.tensor.matmul(
                        ps,
                        xt_sb[:, c, 128 * m:128 * (m + 1)],
                        wv_sb[:, c, :],
                        start=(c == 0), stop=(c == CT - 1),
                    )
                nc.vector.tensor_tensor(
                    out=v_sb[:, m, :], in0=ps, in1=bvb_sb, op=ADD,
                )

            if stage == 1:
                nc.sync.dma_start(out=dbg_h.ap()[w], in_=qk_sb.rearrange("p j n -> p (j n)"))
                continue

            # ---- attention, head-groups software-pipelined; the previous
            # window's out_proj slots between attnv(g1) and attnv(g2) so PE
            # has work while exp(g2) drains ----
            def do_scores(g):
                attn_sb = apool.tile([128, 2 * 4 * N], BF16, name="attn_sb")
                for t in range(TT):
                    for u in range(2):
                        # one PSUM bank per matmul group (HW requirement):
                        # 2-head units of 2 banks so the pool double-buffers
                        scp = sc_ps.tile([128, 2, 512], F32, name="scp")
                        for h2 in range(2):
                            hh = 2 * u + h2
                            # S^T[k_slice, q] = k_h[k_slice] @ q_h^T
                            nc.tensor.matmul(
                                scp[:, h2, 0:N],
                                qk_sb[32 * hh:32 * (hh + 1), CT + g, 128 * t:128 * (t + 1)],
                                qk_sb[32 * hh:32 * (hh + 1), g, :],
                                start=True, stop=True,
                                tile_position=(32 * hh, 0),
                            )
                        nc.scalar.activation(
                            out=attn_sb[:, 4 * N * t + 2 * N * u: 4 * N * t + 2 * N * (u + 1)],
                            in_=scp[:, :, 0:N],
                            func=EXP, scale=SCALE,
                        )
                return attn_sb

            def do_attnv(g, attn_sb):
                # o^T accumulation bank; denominators borrow a scores-pool slot
                od = att_ps.tile([128, 512], F32, name="od")
                dent = sc_ps.tile([128, 512], F32, tag="scp", name="dent")
                for t in range(TT):
                    for hh in range(4):
                        nc.tensor.matmul(
                            dent[32 * hh:32 * (hh + 1), 0:N],
                            ones_sb[:, :],
                            attn_sb[:, 4 * N * t + N * hh: 4 * N * t + N * (hh + 1)],
                            start=(t == 0), stop=(t == TT - 1),
                            tile_position=(0, 32 * hh),
                            skip_group_check=True,
                        )
                recip_sb = rpool.tile([128, N], F32)
                nc.vector.reciprocal_approx_fast(recip_sb, dent[:, 0:N])
                for t in range(TT):
                    for hh in range(4):
                        h = 4 * g + hh
                        nc.tensor.matmul(
                            od[32 * hh:32 * (hh + 1), 0:N],
                            v_sb[:, t, 32 * h:32 * (h + 1)],
                            attn_sb[:, 4 * N * t + N * hh: 4 * N * t + N * (hh + 1)],
                            start=(t == 0), stop=(t == TT - 1),
                            tile_position=(0, 32 * hh),
                            skip_group_check=True,
                        )
                ot_sb = opool.tile([128, N], F32R, name="ot_sb")
                nc.vector.tensor_tensor(out=ot_sb, in0=od[:, 0:N], in1=recip_sb, op=MULT)
                return ot_sb

            def emit_out_proj(ots, ow):
                of_sb = fpool.tile([128, TT, C], F32)
                for m in range(TT):
                    ps = proj_ps.tile([128, C], F32, tag="proj", name="ps_out")
                    for g in range(NG):
                        nc.tensor.matmul(
                            ps,
                            ots[g][:, 128 * m:128 * (m + 1)],
                            wo_sb[:, g, :],
                            start=(g == 0), stop=(g == NG - 1),
                        )
                    if m == 0:
                        nc.scalar.copy(out=of_sb[:, m, :], in_=ps)
                    else:
                        nc.vector.tensor_copy(out=of_sb[:, m, :], in_=ps)
                nc.sync.dma_start(out=out_h.ap()[ow].rearrange("m p c -> p m c"),
                                  in_=of_sb)

            ot_tiles = []
            attn_next = do_scores(0)
            for g in range(NG):
                attn_cur = attn_next
                if g + 1 < NG:
                    attn_next = do_scores(g + 1)
                if g == NG - 1 and prev_ots is not None and stage == 4:
                    emit_out_proj(prev_ots, prev_w)
                if stage == 2:
                    nc.sync.dma_start(out=dbg_h.ap()[w][g], in_=attn_cur)
                    continue
                ot_sb = do_attnv(g, attn_cur)
                ot_tiles.append(ot_sb)
                if stage == 3:
                    nc.sync.dma_start(out=dbg_h.ap()[w][g], in_=ot_sb)

            if stage in (2, 3):
                continue
            prev_ots = ot_tiles
            prev_w = w

        # ---- out_proj of the final window ----
        if stage == 4:
            of_sb = fpool.tile([128, TT, C], F32)
            for m in range(TT):
                ps = proj_ps.tile([128, C], F32, tag="proj", name="ps_out")
                for g in range(NG):
                    nc.tensor.matmul(
                        ps,
                        prev_ots[g][:, 128 * m:128 * (m + 1)],
                        wo_sb[:, g, :],
                        start=(g == 0), stop=(g == NG - 1),
                    )
                if m == 0:
                    nc.scalar.copy(out=of_sb[:, m, :], in_=ps)
                else:
                    nc.vector.tensor_copy(out=of_sb[:, m, :], in_=ps)
            nc.sync.dma_start(out=out_h.ap()[prev_w].rearrange("m p c -> p m c"),
                              in_=of_sb)

    nc.compile()
    return nc


_PROGRAM = None


def _get_program():
    global _PROGRAM
    if _PROGRAM is None:
        _PROGRAM = build_program()
    return _PROGRAM


def make_in_maps(x, in_proj_weight, in_proj_bias, out_proj_weight):
    x = np.asarray(x, dtype=np.float32)
    in_proj_weight = np.asarray(in_proj_weight, dtype=np.float32)
    in_proj_bias = np.asarray(in_proj_bias, dtype=np.float32)
    out_proj_weight = np.asarray(out_proj_weight, dtype=np.float32)

    xt = np.ascontiguousarray(x.reshape(B * NWIN, N, C).transpose(0, 2, 1))
    xt = xt.reshape(NCORES, WPC, CT, 128, N)
    wqkt = np.ascontiguousarray(in_proj_weight[:2 * C].T).reshape(CT, 128, 2 * C)
    wvt = np.ascontiguousarray(in_proj_weight[2 * C:].T).reshape(CT, 128, C)
    wot = np.ascontiguousarray(out_proj_weight.T).reshape(CT, 128, C)
    bqkt = np.ascontiguousarray(np.broadcast_to(
        in_proj_bias[:2 * C].reshape(2 * CT, 128).T[:, :, None],
        (128, 2 * CT, N)).astype(np.float32))
    bvb = np.ascontiguousarray(np.broadcast_to(in_proj_bias[2 * C:], (128, C)))
    return [
        {"xt": xt[i], "wqkt": wqkt, "wvt": wvt, "wot": wot, "bqkt": bqkt, "bvb": bvb}
        for i in range(NCORES)
    ]


def assemble_out(results):
    outs = [r["out"].reshape(WPC, N, C) for r in results]
    return np.concatenate(outs).reshape(B, NWIN, N, C).astype(np.float32)


def kernel(x, in_proj_weight, in_proj_bias, out_proj_weight):
    nc = _get_program()
    in_maps = make_in_maps(x, in_proj_weight, in_proj_bias, out_proj_weight)
    res = run_bass_kernel_spmd(nc, in_maps, core_ids=list(range(NCORES)))
    return assemble_out(res.results)

